# revision 5
# baseline (speedup 1.0000x reference)
"""Trainium2 Bass kernel for nn_ContinuousActor (GNN message passing actor MLP).

Strategy (pure data parallel over 8 cores, batch dim sharded):
  - Host gathers per-pair inputs: each of the 6 pairs needs only 53 of the 74
    input features, so two pairs pack into one 128-partition tile (rows 0:64
    and 64:128). phi1 then runs as ROW-TILED concurrent matmuls (tile_position
    (0,0) and (64,0)) — two pair-matmuls per 512-cycle PE slot.
  - Per-pair phi1 bias (+ one-hot columns) folds into a ones-row of the
    gathered input.
  - phi2/rho as 128x128-chunk matmuls; 6-pair sum-pool done by a single
    strided DVE tensor_reduce; rho bias+relu on ACT; head bias+clip on GPSIMD
    (no bias matmul).
  - All matmuls bf16 (fp8 fails the accuracy budget; measured by numpy probe).
"""

import numpy as np
import ml_dtypes
from contextlib import ExitStack

import concourse.bass as bass
import concourse.mybir as mybir
import concourse.tile as tile
from concourse import bacc
from concourse.bass_utils import run_bass_kernel_spmd

F32 = mybir.dt.float32
BF16 = mybir.dt.bfloat16
RELU = mybir.ActivationFunctionType.Relu
DT_MM = BF16
DT_NP = ml_dtypes.bfloat16

B_FULL = 65536
N_CORES = 8
BC = B_FULL // N_CORES  # 8192 batch rows per core
BT = 512                # batch tile (matmul free dim)
KG = 64                 # rows per gathered pair slot (53 used + zero pad)
NB_OBJ = 3
DIM_BODY = 10
DIM_OBJECT = 15
PERMS = [(0, 1), (0, 2), (1, 0), (1, 2), (2, 0), (2, 1)]
LOG_SIG_MIN, LOG_SIG_MAX = -20.0, 2.0

_CACHE = {}


def _pack_weights(phi_w1, phi_b1, phi_w2, phi_b2, rho_w1, rho_b1,
                  mean_w, mean_b, logstd_w, logstd_b):
    """Host-side weight repacking into device layouts."""
    f = np.float32
    phi_w1 = np.asarray(phi_w1, f)
    # per-pair effective phi1 weights in gathered-row order, row-tiled layout:
    # w1rt[128, 6*128]: col block (2g+m)*128 holds pair 2g in rows 0:53 and
    # pair 2g+1 in rows 64:117 (row 52/116 is the ones/bias row).
    w1rt = np.zeros((128, 6 * 128), dtype=f)
    for p, (i, j) in enumerate(PERMS):
        g, parity = p // 2, p % 2
        r0 = parity * KG
        W = np.zeros((53, 256), dtype=f)
        W[0:10] = phi_w1[12:22]        # body
        W[10:25] = phi_w1[25:40]       # obj_i features
        W[25:40] = phi_w1[43:58]       # obj_j features
        W[40:43] = phi_w1[0:3]         # ag_i
        W[43:46] = phi_w1[3:6]         # ag_j
        W[46:49] = phi_w1[6:9]         # g_i
        W[49:52] = phi_w1[9:12]        # g_j
        W[52] = phi_b1 + phi_w1[22 + i] + phi_w1[40 + j]  # bias + one-hots
        for m in range(2):
            w1rt[r0:r0 + 53, (2 * g + m) * 128:(2 * g + m + 1) * 128] = \
                W[:, m * 128:(m + 1) * 128]
    # phi2 / rho: [128, 4*128] with col block (2k+m)
    def pack_256(w):
        out = np.empty((128, 512), dtype=f)
        for k in range(2):
            for m in range(2):
                out[:, (2 * k + m) * 128:(2 * k + m + 1) * 128] = \
                    w[k * 128:(k + 1) * 128, m * 128:(m + 1) * 128]
        return out
    w2 = pack_256(np.asarray(phi_w2, f))
    wr = pack_256(np.asarray(rho_w1, f))
    b2 = np.asarray(phi_b2, f).reshape(2, 128).T.copy()   # [128, 2], col m
    br = np.asarray(rho_b1, f).reshape(2, 128).T.copy()
    # heads: [128, 16], col block k*8 = Wh[k*128:(k+1)*128, :]
    wh_full = np.concatenate([np.asarray(mean_w, f), np.asarray(logstd_w, f)], axis=1)
    wh = np.concatenate([wh_full[0:128, :], wh_full[128:256, :]], axis=1)  # [128, 16]
    w1rt, w2, wr, wh = (a.astype(DT_NP) for a in (w1rt, w2, wr, wh))
    # heads combined per-partition constants [8, 3]: bias, clip-hi, clip-lo
    big = np.float32(3.0e38)
    hc = np.empty((8, 3), dtype=f)
    hc[0:4, 0] = np.asarray(mean_b, f)
    hc[4:8, 0] = np.asarray(logstd_b, f)
    hc[0:4, 1], hc[4:8, 1] = big, LOG_SIG_MAX
    hc[0:4, 2], hc[4:8, 2] = -big, LOG_SIG_MIN
    return dict(w1=w1rt, w2=w2, b2=b2, wr=wr, br=br, wh=wh, hc=hc)


def _pack_xg(obs, ag, g):
    """Gathered per-pair inputs, row-tiled: xg[128, 3, B]; rows 0:53 = pair 2gi,
    rows 64:117 = pair 2gi+1, in gathered-row order; row 52/116 = ones."""
    B = obs.shape[0]
    obsT = obs.T.astype(DT_NP)   # [55, B]
    agT = ag.T.astype(DT_NP)     # [9, B]
    gT = g.T.astype(DT_NP)       # [9, B]
    xg = np.zeros((128, 3, B), dtype=DT_NP)
    for p, (i, j) in enumerate(PERMS):
        gi, parity = p // 2, p % 2
        r0 = parity * KG
        blk = xg[:, gi]
        blk[r0 + 0:r0 + 10] = obsT[0:10]
        blk[r0 + 10:r0 + 25] = obsT[10 + 15 * i:25 + 15 * i]
        blk[r0 + 25:r0 + 40] = obsT[10 + 15 * j:25 + 15 * j]
        blk[r0 + 40:r0 + 43] = agT[3 * i:3 * i + 3]
        blk[r0 + 43:r0 + 46] = agT[3 * j:3 * j + 3]
        blk[r0 + 46:r0 + 49] = gT[3 * i:3 * i + 3]
        blk[r0 + 49:r0 + 52] = gT[3 * j:3 * j + 3]
        blk[r0 + 52] = np.asarray(1.0, DT_NP)
    return xg


def _build_bass(bc, bt):
    nt = bc // bt
    nc = bacc.Bacc(trn_type="TRN2")

    xg_d = nc.dram_tensor("xg", [128, 3, bc], DT_MM, kind="ExternalInput")
    w1_d = nc.dram_tensor("w1", [128, 6 * 128], DT_MM, kind="ExternalInput")
    w2_d = nc.dram_tensor("w2", [128, 512], DT_MM, kind="ExternalInput")
    b2_d = nc.dram_tensor("b2", [128, 2], F32, kind="ExternalInput")
    wr_d = nc.dram_tensor("wr", [128, 512], DT_MM, kind="ExternalInput")
    br_d = nc.dram_tensor("br", [128, 2], F32, kind="ExternalInput")
    wh_d = nc.dram_tensor("wh", [128, 16], DT_MM, kind="ExternalInput")
    hc_d = nc.dram_tensor("hc", [8, 3], F32, kind="ExternalInput")
    y_d = nc.dram_tensor("y", [8, bc], F32, kind="ExternalOutput")

    AMIN, AMAX, AADD = mybir.AluOpType.min, mybir.AluOpType.max, mybir.AluOpType.add

    with ExitStack() as ctx:
        tc = ctx.enter_context(tile.TileContext(nc))
        consts = ctx.enter_context(tc.tile_pool(name="consts", bufs=1))
        sbp = ctx.enter_context(tc.tile_pool(name="sbp", bufs=2))
        psp = ctx.enter_context(tc.tile_pool(name="psp", bufs=1, space="PSUM"))

        w1sb = consts.tile([128, 6 * 128], DT_MM)
        nc.sync.dma_start(out=w1sb, in_=w1_d[:, :])
        w2sb = consts.tile([128, 512], DT_MM)
        nc.sync.dma_start(out=w2sb, in_=w2_d[:, :])
        wrsb = consts.tile([128, 512], DT_MM)
        nc.sync.dma_start(out=wrsb, in_=wr_d[:, :])
        whsb = consts.tile([128, 16], DT_MM)
        nc.sync.dma_start(out=whsb, in_=wh_d[:, :])
        b2sb = consts.tile([128, 2], F32)
        nc.sync.dma_start(out=b2sb, in_=b2_d[:, :])
        brsb = consts.tile([128, 2], F32)
        nc.sync.dma_start(out=brsb, in_=br_d[:, :])
        hcsb = consts.tile([8, 3], F32)
        nc.sync.dma_start(out=hcsb, in_=hc_d[:, :])

        # engine choice for the 12 per-tile phi2 relus (by (g, pair, m) index).
        # GPSIMD cannot read PSUM, so only ACT/DVE here; ACT also carries the
        # g==1 phi1 relu, DVE the other two plus the pool reduce.
        PH2_ENG = ["act", "dve", "act", "dve", "dve", "act",
                   "dve", "dve", "act", "act", "act", "act"]

        def ph2_relu(eng, out_ap, in_ap, bias_ap):
            if eng == "act":
                nc.scalar.activation(out_ap, in_ap, RELU, bias=bias_ap)
            else:
                nc.vector.tensor_scalar(out_ap, in_ap, bias_ap, 0.0,
                                        op0=AADD, op1=AMAX)

        def finisher(racc, s0):
            state = {}

            def s_pool():
                pooled = sbp.tile([128, 2, 512], DT_MM, tag="pooled", name="pooled")
                with nc.allow_low_precision("6-term pool sum rounds once to bf16"):
                    nc.vector.tensor_reduce(
                        pooled, racc.transpose([0, 1, 3, 2]),
                        axis=mybir.AxisListType.X, op=AADD,
                    )
                state["pooled"] = pooled
                state["xs"] = sbp.tile([128, 2, 512], DT_MM, tag="xs", name="xs")

            def s_rho(m):
                pooled, xs = state["pooled"], state["xs"]
                pr = psp.tile([128, 512], F32, tag="pr", name="pr")
                nc.tensor.matmul(pr, wrsb[:, m * 128:(m + 1) * 128],
                                 pooled[:, 0, :], start=True, stop=False)
                nc.tensor.matmul(pr, wrsb[:, (2 + m) * 128:(3 + m) * 128],
                                 pooled[:, 1, :], start=False, stop=True)
                nc.scalar.activation(xs[:, m, :], pr, RELU,
                                     bias=brsb[:, m:m + 1])

            def s_heads():
                xs = state["xs"]
                py = psp.tile([128, 512], F32, tag="ph2", bufs=3, name="py")
                nc.tensor.matmul(py[0:8, :], whsb[:, 0:8], xs[:, 0, :],
                                 start=True, stop=False)
                nc.tensor.matmul(py[0:8, :], whsb[:, 8:16], xs[:, 1, :],
                                 start=False, stop=True)
                tmp = sbp.tile([8, 512], F32, tag="tmp", name="tmp")
                nc.scalar.activation(tmp, py[0:8, :],
                                     mybir.ActivationFunctionType.Identity,
                                     bias=hcsb[:, 0:1])
                ysb = sbp.tile([8, 512], F32, tag="ysb", name="ysb")
                nc.gpsimd.tensor_scalar(ysb, tmp, hcsb[:, 1:2], hcsb[:, 2:3],
                                        op0=AMIN, op1=AMAX)
                nc.sync.dma_start(out=y_d[:, s0:s0 + 512], in_=ysb)

            return [s_pool, lambda: s_rho(0), lambda: s_rho(1), s_heads]

        pending = []
        for t in range(nt):
            s0 = t * bt
            xgt = sbp.tile([128, 3, bt], DT_MM, tag="xgt", bufs=3)
            nc.sync.dma_start(out=xgt, in_=xg_d[:, :, s0:s0 + bt])
            racc = sbp.tile([128, 2, 6, bt], DT_MM, tag="racc")

            idx = 0  # (g, pair) iteration counter 0..5
            for g in range(3):
                # phi1: two pairs row-tiled into one PE slot per m-chunk
                phg = psp.tile([128, 4, bt], F32, tag="ph1", name="phg")
                for m in range(2):
                    c0 = (2 * g + m) * 128
                    nc.tensor.matmul(phg[:, m, :], w1sb[0:KG, c0:c0 + 128],
                                     xgt[0:KG, g, :], start=True, stop=True)
                    nc.tensor.matmul(phg[:, 2 + m, :], w1sb[KG:128, c0:c0 + 128],
                                     xgt[KG:128, g, :], start=True, stop=True)
                # h slots: 0 = pairA k-chunk0, 1 = pairA k1, 2 = pairB k0, 3 = pairB k1
                h = sbp.tile([128, 4, bt], DT_MM, tag="h")
                if g == 1:
                    nc.scalar.activation(h, phg, RELU)
                else:
                    nc.vector.tensor_scalar_max(h, phg, 0.0)

                for pair in range(2):
                    p = 2 * g + pair
                    hs = 2 * pair
                    for m in range(2):
                        ph2 = psp.tile([128, bt], F32, tag="ph2", bufs=3,
                                       name="ph2")
                        nc.tensor.matmul(ph2, w2sb[:, m * 128:(m + 1) * 128],
                                         h[:, hs, :], start=True, stop=False)
                        nc.tensor.matmul(ph2,
                                         w2sb[:, (2 + m) * 128:(3 + m) * 128],
                                         h[:, hs + 1, :], start=False, stop=True)
                        ph2_relu(PH2_ENG[2 * idx + m], racc[:, m, p, :], ph2,
                                 b2sb[:, m:m + 1])
                    if pending and 0 <= idx - 1 < len(pending):
                        pending[idx - 1]()  # prev tile's pool/rho/heads stages
                    idx += 1
            pending = finisher(racc, s0)
        for stage in pending:
            stage()

    return nc


def _get_nc(bc, bt):
    key = (bc, bt)
    if key not in _CACHE:
        nc = _build_bass(bc, bt)
        nc.finalize()
        _CACHE[key] = nc
    return _CACHE[key]


def kernel(obs, ag, g, phi_w1, phi_b1, phi_w2, phi_b2,
           rho_w1, rho_b1, mean_w, mean_b, logstd_w, logstd_b):
    obs = np.asarray(obs, np.float32)
    ag = np.asarray(ag, np.float32)
    g = np.asarray(g, np.float32)
    B = obs.shape[0]
    assert B == B_FULL, f"kernel hardcoded for B={B_FULL}, got {B}"

    packed = _pack_weights(phi_w1, phi_b1, phi_w2, phi_b2, rho_w1, rho_b1,
                           mean_w, mean_b, logstd_w, logstd_b)
    xg = _pack_xg(obs, ag, g)

    nc = _get_nc(BC, BT)
    in_maps = []
    for c in range(N_CORES):
        m = dict(packed)
        m["xg"] = np.ascontiguousarray(xg[:, :, c * BC:(c + 1) * BC])
        in_maps.append(m)

    import os
    trace = bool(os.environ.get("KERNEL_TRACE"))
    res = run_bass_kernel_spmd(nc, in_maps, core_ids=list(range(N_CORES)),
                               trace=trace)
    global _last_results
    _last_results = res

    y = np.concatenate([res.results[c]["y"] for c in range(N_CORES)], axis=1)  # [8, B]
    out = np.ascontiguousarray(y.T)  # [B, 8]
    mean = out[:, 0:4].copy()
    logstd = out[:, 4:8].copy()
    return mean, logstd


_last_results = None


# revision 8
# speedup vs baseline: 1.0300x; 1.0300x over previous
"""Trainium2 Bass kernel for nn_ContinuousActor (GNN message passing actor MLP).

Strategy (pure data parallel over 8 cores, batch dim sharded):
  - Host gathers per-pair inputs: each of the 6 pairs needs only 53 of the 74
    input features, so two pairs pack into one 128-partition tile (rows 0:64
    and 64:128). phi1 then runs as ROW-TILED concurrent matmuls (tile_position
    (0,0) and (64,0)) — two pair-matmuls per 512-cycle PE slot.
  - Per-pair phi1 bias (+ one-hot columns) folds into a ones-row of the
    gathered input.
  - phi2/rho as 128x128-chunk matmuls; 6-pair sum-pool done by a single
    strided DVE tensor_reduce; rho bias+relu on ACT; head bias+clip on GPSIMD
    (no bias matmul).
  - All matmuls bf16 (fp8 fails the accuracy budget; measured by numpy probe).
"""

import numpy as np
import ml_dtypes
from contextlib import ExitStack

import concourse.bass as bass
import concourse.mybir as mybir
import concourse.tile as tile
from concourse import bacc
from concourse.bass_utils import run_bass_kernel_spmd

F32 = mybir.dt.float32
BF16 = mybir.dt.bfloat16
RELU = mybir.ActivationFunctionType.Relu
DT_MM = BF16
DT_NP = ml_dtypes.bfloat16

B_FULL = 65536
N_CORES = 8
BC = B_FULL // N_CORES  # 8192 batch rows per core
BT = 512                # batch tile (matmul free dim)
KG = 64                 # rows per gathered pair slot (53 used + zero pad)
NB_OBJ = 3
DIM_BODY = 10
DIM_OBJECT = 15
PERMS = [(0, 1), (0, 2), (1, 0), (1, 2), (2, 0), (2, 1)]
LOG_SIG_MIN, LOG_SIG_MAX = -20.0, 2.0

_CACHE = {}


def _pack_weights(phi_w1, phi_b1, phi_w2, phi_b2, rho_w1, rho_b1,
                  mean_w, mean_b, logstd_w, logstd_b):
    """Host-side weight repacking into device layouts."""
    f = np.float32
    phi_w1 = np.asarray(phi_w1, f)
    # per-pair effective phi1 weights in gathered-row order, row-tiled layout:
    # w1rt[128, 6*128]: col block (2g+m)*128 holds pair 2g in rows 0:53 and
    # pair 2g+1 in rows 64:117 (row 52/116 is the ones/bias row).
    w1rt = np.zeros((128, 6 * 128), dtype=f)
    for p, (i, j) in enumerate(PERMS):
        g, parity = p // 2, p % 2
        r0 = parity * KG
        W = np.zeros((53, 256), dtype=f)
        W[0:10] = phi_w1[12:22]        # body
        W[10:25] = phi_w1[25:40]       # obj_i features
        W[25:40] = phi_w1[43:58]       # obj_j features
        W[40:43] = phi_w1[0:3]         # ag_i
        W[43:46] = phi_w1[3:6]         # ag_j
        W[46:49] = phi_w1[6:9]         # g_i
        W[49:52] = phi_w1[9:12]        # g_j
        W[52] = phi_b1 + phi_w1[22 + i] + phi_w1[40 + j]  # bias + one-hots
        for m in range(2):
            w1rt[r0:r0 + 53, (2 * g + m) * 128:(2 * g + m + 1) * 128] = \
                W[:, m * 128:(m + 1) * 128]
    # phi2 / rho: [128, 4*128] with col block (2k+m)
    def pack_256(w):
        out = np.empty((128, 512), dtype=f)
        for k in range(2):
            for m in range(2):
                out[:, (2 * k + m) * 128:(2 * k + m + 1) * 128] = \
                    w[k * 128:(k + 1) * 128, m * 128:(m + 1) * 128]
        return out
    w2 = pack_256(np.asarray(phi_w2, f))
    wr = pack_256(np.asarray(rho_w1, f))
    b2 = np.asarray(phi_b2, f).reshape(2, 128).T.copy()   # [128, 2], col m
    br = np.asarray(rho_b1, f).reshape(2, 128).T.copy()
    # heads: [128, 16], col block k*8 = Wh[k*128:(k+1)*128, :]
    wh_full = np.concatenate([np.asarray(mean_w, f), np.asarray(logstd_w, f)], axis=1)
    wh = np.concatenate([wh_full[0:128, :], wh_full[128:256, :]], axis=1)  # [128, 16]
    w1rt, w2, wr, wh = (a.astype(DT_NP) for a in (w1rt, w2, wr, wh))
    # heads combined per-partition constants [8, 3]: bias, clip-hi, clip-lo
    big = np.float32(3.0e38)
    hc = np.empty((8, 3), dtype=f)
    hc[0:4, 0] = np.asarray(mean_b, f)
    hc[4:8, 0] = np.asarray(logstd_b, f)
    hc[0:4, 1], hc[4:8, 1] = big, LOG_SIG_MAX
    hc[0:4, 2], hc[4:8, 2] = -big, LOG_SIG_MIN
    return dict(w1=w1rt, w2=w2, b2=b2, wr=wr, br=br, wh=wh, hc=hc)


def _pack_xg(obs, ag, g):
    """Gathered per-pair inputs, row-tiled: xg[128, 3, B]; rows 0:53 = pair 2gi,
    rows 64:117 = pair 2gi+1, in gathered-row order; row 52/116 = ones."""
    B = obs.shape[0]
    obsT = obs.T.astype(DT_NP)   # [55, B]
    agT = ag.T.astype(DT_NP)     # [9, B]
    gT = g.T.astype(DT_NP)       # [9, B]
    xg = np.zeros((128, 3, B), dtype=DT_NP)
    for p, (i, j) in enumerate(PERMS):
        gi, parity = p // 2, p % 2
        r0 = parity * KG
        blk = xg[:, gi]
        blk[r0 + 0:r0 + 10] = obsT[0:10]
        blk[r0 + 10:r0 + 25] = obsT[10 + 15 * i:25 + 15 * i]
        blk[r0 + 25:r0 + 40] = obsT[10 + 15 * j:25 + 15 * j]
        blk[r0 + 40:r0 + 43] = agT[3 * i:3 * i + 3]
        blk[r0 + 43:r0 + 46] = agT[3 * j:3 * j + 3]
        blk[r0 + 46:r0 + 49] = gT[3 * i:3 * i + 3]
        blk[r0 + 49:r0 + 52] = gT[3 * j:3 * j + 3]
        blk[r0 + 52] = np.asarray(1.0, DT_NP)
    return xg


def _build_bass(bc, bt):
    nt = bc // bt
    nc = bacc.Bacc(trn_type="TRN2")

    xg_d = nc.dram_tensor("xg", [128, 3, bc], DT_MM, kind="ExternalInput")
    w1_d = nc.dram_tensor("w1", [128, 6 * 128], DT_MM, kind="ExternalInput")
    w2_d = nc.dram_tensor("w2", [128, 512], DT_MM, kind="ExternalInput")
    b2_d = nc.dram_tensor("b2", [128, 2], F32, kind="ExternalInput")
    wr_d = nc.dram_tensor("wr", [128, 512], DT_MM, kind="ExternalInput")
    br_d = nc.dram_tensor("br", [128, 2], F32, kind="ExternalInput")
    wh_d = nc.dram_tensor("wh", [128, 16], DT_MM, kind="ExternalInput")
    hc_d = nc.dram_tensor("hc", [8, 3], F32, kind="ExternalInput")
    y_d = nc.dram_tensor("y", [8, bc], F32, kind="ExternalOutput")

    AMIN, AMAX, AADD = mybir.AluOpType.min, mybir.AluOpType.max, mybir.AluOpType.add

    with ExitStack() as ctx:
        tc = ctx.enter_context(tile.TileContext(nc))
        consts = ctx.enter_context(tc.tile_pool(name="consts", bufs=1))
        sbp = ctx.enter_context(tc.tile_pool(name="sbp", bufs=2))
        psp = ctx.enter_context(tc.tile_pool(name="psp", bufs=1, space="PSUM"))

        w1sb = consts.tile([128, 6 * 128], DT_MM)
        nc.sync.dma_start(out=w1sb, in_=w1_d[:, :])
        w2sb = consts.tile([128, 512], DT_MM)
        nc.sync.dma_start(out=w2sb, in_=w2_d[:, :])
        wrsb = consts.tile([128, 512], DT_MM)
        nc.sync.dma_start(out=wrsb, in_=wr_d[:, :])
        whsb = consts.tile([128, 16], DT_MM)
        nc.sync.dma_start(out=whsb, in_=wh_d[:, :])
        b2sb = consts.tile([128, 2], F32)
        nc.sync.dma_start(out=b2sb, in_=b2_d[:, :])
        brsb = consts.tile([128, 2], F32)
        nc.sync.dma_start(out=brsb, in_=br_d[:, :])
        hcsb = consts.tile([8, 3], F32)
        nc.sync.dma_start(out=hcsb, in_=hc_d[:, :])

        # engine choice for the 12 per-tile phi2 relus (by (g, pair, m) index).
        # GPSIMD cannot read PSUM, so only ACT/DVE here (measured ~100-133G/s
        # each on PSUM reads); ACT also carries the g==1 phi1 relu and rho,
        # DVE the other two phi1 relus; GPSIMD gets the SBUF-only pool adds.
        PH2_ENG = ["act", "dve", "act", "act", "dve", "act",
                   "act", "dve", "act", "act", "dve", "act"]

        def ph2_relu(eng, out_ap, in_ap, bias_ap):
            if eng == "act":
                nc.scalar.activation(out_ap, in_ap, RELU, bias=bias_ap)
            else:
                nc.vector.tensor_scalar(out_ap, in_ap, bias_ap, 0.0,
                                        op0=AADD, op1=AMAX)

        def finisher(pooled, s0):
            state = {}

            def s_rho(m):
                if "xs" not in state:
                    state["xs"] = sbp.tile([128, 2, 512], DT_MM, tag="xs",
                                           name="xs")
                xs = state["xs"]
                pr = psp.tile([128, 512], F32, tag="pr", name="pr")
                nc.tensor.matmul(pr, wrsb[:, m * 128:(m + 1) * 128],
                                 pooled[:, 0, :], start=True, stop=False)
                nc.tensor.matmul(pr, wrsb[:, (2 + m) * 128:(3 + m) * 128],
                                 pooled[:, 1, :], start=False, stop=True)
                nc.scalar.activation(xs[:, m, :], pr, RELU,
                                     bias=brsb[:, m:m + 1])

            def s_heads():
                xs = state["xs"]
                py = psp.tile([128, 512], F32, tag="ph2", bufs=3, name="py")
                nc.tensor.matmul(py[0:8, :], whsb[:, 0:8], xs[:, 0, :],
                                 start=True, stop=False)
                nc.tensor.matmul(py[0:8, :], whsb[:, 8:16], xs[:, 1, :],
                                 start=False, stop=True)
                tmp = sbp.tile([8, 512], F32, tag="tmp", name="tmp")
                nc.scalar.activation(tmp, py[0:8, :],
                                     mybir.ActivationFunctionType.Identity,
                                     bias=hcsb[:, 0:1])
                ysb = sbp.tile([8, 512], F32, tag="ysb", name="ysb")
                nc.gpsimd.tensor_scalar(ysb, tmp, hcsb[:, 1:2], hcsb[:, 2:3],
                                        op0=AMIN, op1=AMAX)
                nc.sync.dma_start(out=y_d[:, s0:s0 + 512], in_=ysb)

            return [lambda: s_rho(0), lambda: s_rho(1), s_heads]

        pending = []
        for t in range(nt):
            s0 = t * bt
            xgt = sbp.tile([128, 3, bt], DT_MM, tag="xgt", bufs=3)
            nc.sync.dma_start(out=xgt, in_=xg_d[:, :, s0:s0 + bt])
            # per-pair phi2 relu outputs: pair 0 -> acc, pairs 1-5 -> rs[p]
            acc = sbp.tile([128, 2, bt], DT_MM, tag="acc")
            rs = [None] * 6
            for p in range(1, 6):
                rs[p] = sbp.tile([128, 2, bt], DT_MM, tag=f"r{p}", name=f"r{p}")
            ts_ = [None, None]
            final = sbp.tile([128, 2, bt], DT_MM, tag="final")

            idx = 0  # (g, pair) iteration counter 0..5
            for g in range(3):
                # phi1: two pairs row-tiled into one PE slot per m-chunk
                phg = psp.tile([128, 4, bt], F32, tag="ph1", name="phg")
                for m in range(2):
                    c0 = (2 * g + m) * 128
                    nc.tensor.matmul(phg[:, m, :], w1sb[0:KG, c0:c0 + 128],
                                     xgt[0:KG, g, :], start=True, stop=True)
                    nc.tensor.matmul(phg[:, 2 + m, :], w1sb[KG:128, c0:c0 + 128],
                                     xgt[KG:128, g, :], start=True, stop=True)
                # h slots: 0 = pairA k-chunk0, 1 = pairA k1, 2 = pairB k0, 3 = pairB k1
                h = sbp.tile([128, 4, bt], DT_MM, tag="h")
                if g == 1:
                    nc.scalar.activation(h, phg, RELU)
                else:
                    nc.vector.tensor_scalar_max(h, phg, 0.0)

                for pair in range(2):
                    p = 2 * g + pair
                    hs = 2 * pair
                    dst = acc if p == 0 else rs[p]
                    for m in range(2):
                        ph2 = psp.tile([128, bt], F32, tag="ph2", bufs=3,
                                       name="ph2")
                        nc.tensor.matmul(ph2, w2sb[:, m * 128:(m + 1) * 128],
                                         h[:, hs, :], start=True, stop=False)
                        nc.tensor.matmul(ph2,
                                         w2sb[:, (2 + m) * 128:(3 + m) * 128],
                                         h[:, hs + 1, :], start=False, stop=True)
                        ph2_relu(PH2_ENG[2 * idx + m], dst[:, m, :], ph2,
                                 b2sb[:, m:m + 1])
                    # pool-add tree on GPSIMD (SBUF-only) as pair outputs land
                    if p == 2:
                        ts_[0] = sbp.tile([128, 2, bt], DT_MM, tag="t0",
                                          name="t0")
                        nc.gpsimd.tensor_add(ts_[0], rs[1], rs[2])
                    elif p == 4:
                        ts_[1] = sbp.tile([128, 2, bt], DT_MM, tag="t1",
                                          name="t1")
                        nc.gpsimd.tensor_add(ts_[1], rs[3], rs[4])
                    elif p == 5:
                        nc.gpsimd.tensor_add(ts_[0], ts_[0], ts_[1])
                        nc.vector.tensor_add(rs[5], rs[5], acc)
                        nc.vector.tensor_add(final, rs[5], ts_[0])
                    if pending and 0 <= idx - 1 < len(pending):
                        pending[idx - 1]()  # prev tile's rho/heads stages
                    idx += 1
            pending = finisher(final, s0)
        for stage in pending:
            stage()

    return nc


def _get_nc(bc, bt):
    key = (bc, bt)
    if key not in _CACHE:
        nc = _build_bass(bc, bt)
        nc.finalize()
        _CACHE[key] = nc
    return _CACHE[key]


def kernel(obs, ag, g, phi_w1, phi_b1, phi_w2, phi_b2,
           rho_w1, rho_b1, mean_w, mean_b, logstd_w, logstd_b):
    obs = np.asarray(obs, np.float32)
    ag = np.asarray(ag, np.float32)
    g = np.asarray(g, np.float32)
    B = obs.shape[0]
    assert B == B_FULL, f"kernel hardcoded for B={B_FULL}, got {B}"

    packed = _pack_weights(phi_w1, phi_b1, phi_w2, phi_b2, rho_w1, rho_b1,
                           mean_w, mean_b, logstd_w, logstd_b)
    xg = _pack_xg(obs, ag, g)

    nc = _get_nc(BC, BT)
    in_maps = []
    for c in range(N_CORES):
        m = dict(packed)
        m["xg"] = np.ascontiguousarray(xg[:, :, c * BC:(c + 1) * BC])
        in_maps.append(m)

    import os
    trace = bool(os.environ.get("KERNEL_TRACE"))
    res = run_bass_kernel_spmd(nc, in_maps, core_ids=list(range(N_CORES)),
                               trace=trace)
    global _last_results
    _last_results = res

    y = np.concatenate([res.results[c]["y"] for c in range(N_CORES)], axis=1)  # [8, B]
    out = np.ascontiguousarray(y.T)  # [B, 8]
    mean = out[:, 0:4].copy()
    logstd = out[:, 4:8].copy()
    return mean, logstd


_last_results = None


# revision 12
# speedup vs baseline: 1.1900x; 1.1554x over previous
"""Trainium2 Bass kernel for nn_ContinuousActor (GNN message passing actor MLP).

Strategy (pure data parallel over 8 cores, batch dim sharded):
  - Host gathers per-pair inputs: each of the 6 pairs needs only 53 of the 74
    input features, so two pairs pack into one 128-partition tile (rows 0:64
    and 64:128). phi1 then runs as ROW-TILED concurrent matmuls (tile_position
    (0,0) and (64,0)) — two pair-matmuls per 512-cycle PE slot.
  - Per-pair phi1 bias (+ one-hot columns) folds into a ones-row of the
    gathered input.
  - phi2/rho as 128x128-chunk matmuls; 6-pair sum-pool done by a single
    strided DVE tensor_reduce; rho bias+relu on ACT; head bias+clip on GPSIMD
    (no bias matmul).
  - All matmuls bf16 (fp8 fails the accuracy budget; measured by numpy probe).
"""

import numpy as np
import ml_dtypes
from contextlib import ExitStack

import concourse.bass as bass
import concourse.mybir as mybir
import concourse.tile as tile
from concourse import bacc
from concourse.bass_utils import run_bass_kernel_spmd

F32 = mybir.dt.float32
BF16 = mybir.dt.bfloat16
RELU = mybir.ActivationFunctionType.Relu
DT_MM = BF16
DT_NP = ml_dtypes.bfloat16

B_FULL = 65536
N_CORES = 8
BC = B_FULL // N_CORES  # 8192 batch rows per core
BT = 512                # batch tile (matmul free dim)
KG = 64                 # rows per gathered pair slot (53 used + zero pad)
NB_OBJ = 3
DIM_BODY = 10
DIM_OBJECT = 15
PERMS = [(0, 1), (0, 2), (1, 0), (1, 2), (2, 0), (2, 1)]
LOG_SIG_MIN, LOG_SIG_MAX = -20.0, 2.0

_CACHE = {}


def _pack_weights(phi_w1, phi_b1, phi_w2, phi_b2, rho_w1, rho_b1,
                  mean_w, mean_b, logstd_w, logstd_b):
    """Host-side weight repacking into device layouts."""
    f = np.float32
    phi_w1 = np.asarray(phi_w1, f)
    # per-pair effective phi1 weights in gathered-row order, row-tiled layout:
    # w1rt[128, 6*128]: col block (2g+m)*128 holds pair 2g in rows 0:53 and
    # pair 2g+1 in rows 64:117 (row 52/116 is the ones/bias row).
    w1rt = np.zeros((128, 6 * 128), dtype=f)
    for p, (i, j) in enumerate(PERMS):
        g, parity = p // 2, p % 2
        r0 = parity * KG
        W = np.zeros((53, 256), dtype=f)
        W[0:10] = phi_w1[12:22]        # body
        W[10:25] = phi_w1[25:40]       # obj_i features
        W[25:40] = phi_w1[43:58]       # obj_j features
        W[40:43] = phi_w1[0:3]         # ag_i
        W[43:46] = phi_w1[3:6]         # ag_j
        W[46:49] = phi_w1[6:9]         # g_i
        W[49:52] = phi_w1[9:12]        # g_j
        W[52] = phi_b1 + phi_w1[22 + i] + phi_w1[40 + j]  # bias + one-hots
        for m in range(2):
            w1rt[r0:r0 + 53, (2 * g + m) * 128:(2 * g + m + 1) * 128] = \
                W[:, m * 128:(m + 1) * 128]
    # phi2 / rho: [128, 4*128] with col block (2k+m)
    def pack_256(w):
        out = np.empty((128, 512), dtype=f)
        for k in range(2):
            for m in range(2):
                out[:, (2 * k + m) * 128:(2 * k + m + 1) * 128] = \
                    w[k * 128:(k + 1) * 128, m * 128:(m + 1) * 128]
        return out
    w2 = pack_256(np.asarray(phi_w2, f))
    wr = pack_256(np.asarray(rho_w1, f))
    b2 = np.asarray(phi_b2, f).reshape(2, 128).T.copy()   # [128, 2], col m
    br = np.asarray(rho_b1, f).reshape(2, 128).T.copy()
    # heads: [128, 16], col block k*8 = Wh[k*128:(k+1)*128, :]
    wh_full = np.concatenate([np.asarray(mean_w, f), np.asarray(logstd_w, f)], axis=1)
    wh = np.concatenate([wh_full[0:128, :], wh_full[128:256, :]], axis=1)  # [128, 16]
    w1rt, w2, wr, wh = (a.astype(DT_NP) for a in (w1rt, w2, wr, wh))
    # heads combined per-partition constants [8, 3]: bias, clip-hi, clip-lo
    big = np.float32(3.0e38)
    hc = np.empty((8, 3), dtype=f)
    hc[0:4, 0] = np.asarray(mean_b, f)
    hc[4:8, 0] = np.asarray(logstd_b, f)
    hc[0:4, 1], hc[4:8, 1] = big, LOG_SIG_MAX
    hc[0:4, 2], hc[4:8, 2] = -big, LOG_SIG_MIN
    return dict(w1=w1rt, w2=w2, b2=b2, wr=wr, br=br, wh=wh, hc=hc)


def _pack_xg(obs, ag, g):
    """Gathered per-pair inputs, row-tiled: xg[128, 3, B]; rows 0:53 = pair 2gi,
    rows 64:117 = pair 2gi+1, in gathered-row order; row 52/116 = ones."""
    B = obs.shape[0]
    obsT = obs.T.astype(DT_NP)   # [55, B]
    agT = ag.T.astype(DT_NP)     # [9, B]
    gT = g.T.astype(DT_NP)       # [9, B]
    xg = np.zeros((128, 3, B), dtype=DT_NP)
    for p, (i, j) in enumerate(PERMS):
        gi, parity = p // 2, p % 2
        r0 = parity * KG
        blk = xg[:, gi]
        blk[r0 + 0:r0 + 10] = obsT[0:10]
        blk[r0 + 10:r0 + 25] = obsT[10 + 15 * i:25 + 15 * i]
        blk[r0 + 25:r0 + 40] = obsT[10 + 15 * j:25 + 15 * j]
        blk[r0 + 40:r0 + 43] = agT[3 * i:3 * i + 3]
        blk[r0 + 43:r0 + 46] = agT[3 * j:3 * j + 3]
        blk[r0 + 46:r0 + 49] = gT[3 * i:3 * i + 3]
        blk[r0 + 49:r0 + 52] = gT[3 * j:3 * j + 3]
        blk[r0 + 52] = np.asarray(1.0, DT_NP)
    return xg


def _build_bass(bc, bt):
    nt = bc // bt
    nc = bacc.Bacc(trn_type="TRN2")

    xg_d = nc.dram_tensor("xg", [128, 3, bc], DT_MM, kind="ExternalInput")
    w1_d = nc.dram_tensor("w1", [128, 6 * 128], DT_MM, kind="ExternalInput")
    w2_d = nc.dram_tensor("w2", [128, 512], DT_MM, kind="ExternalInput")
    b2_d = nc.dram_tensor("b2", [128, 2], F32, kind="ExternalInput")
    wr_d = nc.dram_tensor("wr", [128, 512], DT_MM, kind="ExternalInput")
    br_d = nc.dram_tensor("br", [128, 2], F32, kind="ExternalInput")
    wh_d = nc.dram_tensor("wh", [128, 16], DT_MM, kind="ExternalInput")
    hc_d = nc.dram_tensor("hc", [8, 3], F32, kind="ExternalInput")
    y_d = nc.dram_tensor("y", [8, bc], F32, kind="ExternalOutput")

    AMIN, AMAX, AADD = mybir.AluOpType.min, mybir.AluOpType.max, mybir.AluOpType.add

    with ExitStack() as ctx:
        tc = ctx.enter_context(tile.TileContext(nc))
        consts = ctx.enter_context(tc.tile_pool(name="consts", bufs=1))
        sbp = ctx.enter_context(tc.tile_pool(name="sbp", bufs=2))
        psp = ctx.enter_context(tc.tile_pool(name="psp", bufs=1, space="PSUM"))

        w1sb = consts.tile([128, 6 * 128], DT_MM)
        nc.sync.dma_start(out=w1sb, in_=w1_d[:, :])
        w2sb = consts.tile([128, 512], DT_MM)
        nc.sync.dma_start(out=w2sb, in_=w2_d[:, :])
        wrsb = consts.tile([128, 512], DT_MM)
        nc.sync.dma_start(out=wrsb, in_=wr_d[:, :])
        whsb = consts.tile([128, 16], DT_MM)
        nc.sync.dma_start(out=whsb, in_=wh_d[:, :])
        b2sb = consts.tile([128, 2], F32)
        nc.sync.dma_start(out=b2sb, in_=b2_d[:, :])
        brsb = consts.tile([128, 2], F32)
        nc.sync.dma_start(out=brsb, in_=br_d[:, :])
        hcsb = consts.tile([8, 3], F32)
        nc.sync.dma_start(out=hcsb, in_=hc_d[:, :])

        # GPSIMD cannot read PSUM, so phi1/phi2/rho relus go to ACT/DVE
        # (measured ~100-133G/s each on PSUM reads); GPSIMD gets the
        # SBUF-only pool adds and the final clip.
        def ph2_relu(eng, out_ap, in_ap, bias_ap):
            if eng == "act":
                nc.scalar.activation(out_ap, in_ap, RELU, bias=bias_ap)
            else:
                nc.vector.tensor_scalar(out_ap, in_ap, bias_ap, 0.0,
                                        op0=AADD, op1=AMAX)

        def finisher(pooled, s0):
            state = {}

            def s_rho(m):
                if "xs" not in state:
                    state["xs"] = sbp.tile([128, 2, 512], DT_MM, tag="xs",
                                           name="xs")
                xs = state["xs"]
                pr = psp.tile([128, 512], F32, tag="ph2", bufs=2, name="pr")
                nc.tensor.matmul(pr, wrsb[:, m * 128:(m + 1) * 128],
                                 pooled[:, 0, :], start=True, stop=False)
                nc.tensor.matmul(pr, wrsb[:, (2 + m) * 128:(3 + m) * 128],
                                 pooled[:, 1, :], start=False, stop=True)
                nc.scalar.activation(xs[:, m, :], pr, RELU,
                                     bias=brsb[:, m:m + 1])

            def s_heads():
                xs = state["xs"]
                py = psp.tile([128, 512], F32, tag="ph2", bufs=2, name="py")
                nc.tensor.matmul(py[0:8, :], whsb[:, 0:8], xs[:, 0, :],
                                 start=True, stop=False)
                nc.tensor.matmul(py[0:8, :], whsb[:, 8:16], xs[:, 1, :],
                                 start=False, stop=True)
                tmp = sbp.tile([8, 512], F32, tag="tmp", name="tmp")
                nc.scalar.activation(tmp, py[0:8, :],
                                     mybir.ActivationFunctionType.Identity,
                                     bias=hcsb[:, 0:1])
                ysb = sbp.tile([8, 512], F32, tag="ysb", name="ysb")
                nc.gpsimd.tensor_scalar(ysb, tmp, hcsb[:, 1:2], hcsb[:, 2:3],
                                        op0=AMIN, op1=AMAX)
                nc.sync.dma_start(out=y_d[:, s0:s0 + 512], in_=ysb)

            return [lambda: s_rho(0), lambda: s_rho(1), s_heads]

        pending = []
        for t in range(nt):
            s0 = t * bt
            xgt = sbp.tile([128, 3, bt], DT_MM, tag="xgt", bufs=3)
            nc.sync.dma_start(out=xgt, in_=xg_d[:, :, s0:s0 + bt])
            # rg[g]: phi2 relu outputs [pair, m, bt]; sg[g]: per-g pair sums
            rg = [None] * 3
            sg = [None] * 3
            hh = [None] * 3
            final = sbp.tile([128, 2, bt], DT_MM, tag="final")

            def phi1(g):
                # two pairs row-tiled into one PE slot per m-chunk; per-pair
                # relus split across ACT/DVE so neither blocks the other
                phg = psp.tile([128, 4, bt], F32, tag="ph1", name="phg")
                for m in range(2):
                    c0 = (2 * g + m) * 128
                    nc.tensor.matmul(phg[:, m, :], w1sb[0:KG, c0:c0 + 128],
                                     xgt[0:KG, g, :], start=True, stop=True)
                    nc.tensor.matmul(phg[:, 2 + m, :], w1sb[KG:128, c0:c0 + 128],
                                     xgt[KG:128, g, :], start=True, stop=True)
                # h slots: 0 = pairA k-chunk0, 1 = pairA k1, 2 = pairB k0/k1
                h = sbp.tile([128, 4, bt], DT_MM, tag="h")
                nc.scalar.activation(h[:, 0:2, :], phg[:, 0:2, :], RELU)
                nc.vector.tensor_scalar_max(h[:, 2:4, :], phg[:, 2:4, :], 0.0)
                hh[g] = h
                rg[g] = sbp.tile([128, 2, 2, bt], DT_MM, tag="rg",
                                 name="rg", bufs=3)

            def phi2(g, m, eng):
                # both pairs' m-chunks into one 2-bank psum tile -> one relu
                h = hh[g]
                ph2 = psp.tile([128, 2, bt], F32, tag="ph2", bufs=2,
                               name="ph2")
                for pair in range(2):
                    nc.tensor.matmul(ph2[:, pair, :],
                                     w2sb[:, m * 128:(m + 1) * 128],
                                     h[:, 2 * pair, :], start=True, stop=False)
                    nc.tensor.matmul(ph2[:, pair, :],
                                     w2sb[:, (2 + m) * 128:(3 + m) * 128],
                                     h[:, 2 * pair + 1, :],
                                     start=False, stop=True)
                ph2_relu(eng, rg[g][:, :, m, :], ph2, b2sb[:, m:m + 1])

            def add_sg(g, eng):
                sg[g] = sbp.tile([128, 2, bt], DT_MM, tag="sg", name="sg",
                                 bufs=3)
                eng.tensor_add(sg[g], rg[g][:, 0, :, :], rg[g][:, 1, :, :])

            def pend(i):
                if pending and i < len(pending):
                    pending[i]()

            # PE-ordered emission: every phi2 block is covered by independent
            # MM work (next phi1 round / prev tile's rho+heads) while its
            # input relu completes.
            phi1(0)
            pend(0)                    # prev rho m0
            pend(1)                    # prev rho m1
            phi2(0, 0, "act")
            phi1(1)
            pend(2)                    # prev heads
            phi2(0, 1, "dve")
            add_sg(0, nc.gpsimd)
            phi2(1, 0, "act")
            phi1(2)
            phi2(1, 1, "dve")
            add_sg(1, nc.gpsimd)
            t01 = sbp.tile([128, 2, bt], DT_MM, tag="t01", name="t01")
            nc.gpsimd.tensor_add(t01, sg[0], sg[1])
            phi2(2, 0, "dve")
            phi2(2, 1, "act")
            add_sg(2, nc.vector)
            nc.vector.tensor_add(final, t01, sg[2])
            pending = finisher(final, s0)
        for stage in pending:
            stage()

    return nc


def _get_nc(bc, bt):
    key = (bc, bt)
    if key not in _CACHE:
        nc = _build_bass(bc, bt)
        nc.finalize()
        _CACHE[key] = nc
    return _CACHE[key]


def kernel(obs, ag, g, phi_w1, phi_b1, phi_w2, phi_b2,
           rho_w1, rho_b1, mean_w, mean_b, logstd_w, logstd_b):
    obs = np.asarray(obs, np.float32)
    ag = np.asarray(ag, np.float32)
    g = np.asarray(g, np.float32)
    B = obs.shape[0]
    assert B == B_FULL, f"kernel hardcoded for B={B_FULL}, got {B}"

    packed = _pack_weights(phi_w1, phi_b1, phi_w2, phi_b2, rho_w1, rho_b1,
                           mean_w, mean_b, logstd_w, logstd_b)
    xg = _pack_xg(obs, ag, g)

    nc = _get_nc(BC, BT)
    in_maps = []
    for c in range(N_CORES):
        m = dict(packed)
        m["xg"] = np.ascontiguousarray(xg[:, :, c * BC:(c + 1) * BC])
        in_maps.append(m)

    import os
    trace = bool(os.environ.get("KERNEL_TRACE"))
    res = run_bass_kernel_spmd(nc, in_maps, core_ids=list(range(N_CORES)),
                               trace=trace)
    global _last_results
    _last_results = res

    y = np.concatenate([res.results[c]["y"] for c in range(N_CORES)], axis=1)  # [8, B]
    out = np.ascontiguousarray(y.T)  # [B, 8]
    mean = out[:, 0:4].copy()
    logstd = out[:, 4:8].copy()
    return mean, logstd


_last_results = None


# revision 16
# speedup vs baseline: 1.3398x; 1.1258x over previous
"""Trainium2 Bass kernel for nn_ContinuousActor (GNN message passing actor MLP).

Strategy (pure data parallel over 8 cores, batch dim sharded):
  - Host gathers per-pair inputs: each of the 6 pairs needs only 53 of the 74
    input features, so two pairs pack into one 128-partition tile (rows 0:64
    and 64:128). phi1 then runs as ROW-TILED concurrent matmuls (tile_position
    (0,0) and (64,0)) — two pair-matmuls per 512-cycle PE slot.
  - Per-pair phi1 bias (+ one-hot columns) folds into a ones-row of the
    gathered input.
  - phi2/rho as 128x128-chunk matmuls; 6-pair sum-pool done by a single
    strided DVE tensor_reduce; rho bias+relu on ACT; head bias+clip on GPSIMD
    (no bias matmul).
  - All matmuls bf16 (fp8 fails the accuracy budget; measured by numpy probe).
"""

import numpy as np
import ml_dtypes
from contextlib import ExitStack

import concourse.bass as bass
import concourse.mybir as mybir
import concourse.tile as tile
from concourse import bacc
from concourse.bass_utils import run_bass_kernel_spmd

F32 = mybir.dt.float32
BF16 = mybir.dt.bfloat16
RELU = mybir.ActivationFunctionType.Relu
DT_MM = BF16
DT_NP = ml_dtypes.bfloat16

B_FULL = 65536
N_CORES = 8
BC = B_FULL // N_CORES  # 8192 batch rows per core
BT = 512                # batch tile (matmul free dim)
KG = 64                 # rows per gathered pair slot (53 used + zero pad)
NB_OBJ = 3
DIM_BODY = 10
DIM_OBJECT = 15
PERMS = [(0, 1), (0, 2), (1, 0), (1, 2), (2, 0), (2, 1)]
LOG_SIG_MIN, LOG_SIG_MAX = -20.0, 2.0

_CACHE = {}


def _pack_weights(phi_w1, phi_b1, phi_w2, phi_b2, rho_w1, rho_b1,
                  mean_w, mean_b, logstd_w, logstd_b):
    """Host-side weight repacking into device layouts."""
    f = np.float32
    phi_w1 = np.asarray(phi_w1, f)
    # per-pair effective phi1 weights in gathered-row order, row-tiled layout:
    # w1rt[128, 6*128]: col block (2g+m)*128 holds pair 2g in rows 0:53 and
    # pair 2g+1 in rows 64:117 (row 52/116 is the ones/bias row).
    w1rt = np.zeros((128, 6 * 128), dtype=f)
    for p, (i, j) in enumerate(PERMS):
        g, parity = p // 2, p % 2
        r0 = parity * KG
        W = np.zeros((53, 256), dtype=f)
        W[0:10] = phi_w1[12:22]        # body
        W[10:25] = phi_w1[25:40]       # obj_i features
        W[25:40] = phi_w1[43:58]       # obj_j features
        W[40:43] = phi_w1[0:3]         # ag_i
        W[43:46] = phi_w1[3:6]         # ag_j
        W[46:49] = phi_w1[6:9]         # g_i
        W[49:52] = phi_w1[9:12]        # g_j
        W[52] = phi_b1 + phi_w1[22 + i] + phi_w1[40 + j]  # bias + one-hots
        for m in range(2):
            w1rt[r0:r0 + 53, (2 * g + m) * 128:(2 * g + m + 1) * 128] = \
                W[:, m * 128:(m + 1) * 128]
    # phi2 / rho: [128, 4*128] with col block (2k+m)
    def pack_256(w):
        out = np.empty((128, 512), dtype=f)
        for k in range(2):
            for m in range(2):
                out[:, (2 * k + m) * 128:(2 * k + m + 1) * 128] = \
                    w[k * 128:(k + 1) * 128, m * 128:(m + 1) * 128]
        return out
    w2 = pack_256(np.asarray(phi_w2, f))
    wr = pack_256(np.asarray(rho_w1, f))
    b2 = np.asarray(phi_b2, f).reshape(2, 128).T.copy()   # [128, 2], col m
    br = np.asarray(rho_b1, f).reshape(2, 128).T.copy()
    # heads: [128, 16], col block k*8 = Wh[k*128:(k+1)*128, :]
    wh_full = np.concatenate([np.asarray(mean_w, f), np.asarray(logstd_w, f)], axis=1)
    wh = np.concatenate([wh_full[0:128, :], wh_full[128:256, :]], axis=1)  # [128, 16]
    w1rt, w2, wr, wh = (a.astype(DT_NP) for a in (w1rt, w2, wr, wh))
    # heads combined per-partition constants [8, 3]: bias, clip-hi, clip-lo
    big = np.float32(3.0e38)
    hc = np.empty((8, 3), dtype=f)
    hc[0:4, 0] = np.asarray(mean_b, f)
    hc[4:8, 0] = np.asarray(logstd_b, f)
    hc[0:4, 1], hc[4:8, 1] = big, LOG_SIG_MAX
    hc[0:4, 2], hc[4:8, 2] = -big, LOG_SIG_MIN
    return dict(w1=w1rt, w2=w2, b2=b2, wr=wr, br=br, wh=wh, hc=hc)


def _pack_xg(obs, ag, g):
    """Gathered per-pair inputs, row-tiled: xg[128, 3, B]; rows 0:53 = pair 2gi,
    rows 64:117 = pair 2gi+1, in gathered-row order; row 52/116 = ones."""
    B = obs.shape[0]
    obsT = obs.T.astype(DT_NP)   # [55, B]
    agT = ag.T.astype(DT_NP)     # [9, B]
    gT = g.T.astype(DT_NP)       # [9, B]
    xg = np.zeros((128, 3, B), dtype=DT_NP)
    for p, (i, j) in enumerate(PERMS):
        gi, parity = p // 2, p % 2
        r0 = parity * KG
        blk = xg[:, gi]
        blk[r0 + 0:r0 + 10] = obsT[0:10]
        blk[r0 + 10:r0 + 25] = obsT[10 + 15 * i:25 + 15 * i]
        blk[r0 + 25:r0 + 40] = obsT[10 + 15 * j:25 + 15 * j]
        blk[r0 + 40:r0 + 43] = agT[3 * i:3 * i + 3]
        blk[r0 + 43:r0 + 46] = agT[3 * j:3 * j + 3]
        blk[r0 + 46:r0 + 49] = gT[3 * i:3 * i + 3]
        blk[r0 + 49:r0 + 52] = gT[3 * j:3 * j + 3]
        blk[r0 + 52] = np.asarray(1.0, DT_NP)
    return xg


def _build_bass(bc, bt):
    nt = bc // bt
    nc = bacc.Bacc(trn_type="TRN2")

    xg_d = nc.dram_tensor("xg", [128, 3, bc], DT_MM, kind="ExternalInput")
    w1_d = nc.dram_tensor("w1", [128, 6 * 128], DT_MM, kind="ExternalInput")
    w2_d = nc.dram_tensor("w2", [128, 512], DT_MM, kind="ExternalInput")
    b2_d = nc.dram_tensor("b2", [128, 2], F32, kind="ExternalInput")
    wr_d = nc.dram_tensor("wr", [128, 512], DT_MM, kind="ExternalInput")
    br_d = nc.dram_tensor("br", [128, 2], F32, kind="ExternalInput")
    wh_d = nc.dram_tensor("wh", [128, 16], DT_MM, kind="ExternalInput")
    hc_d = nc.dram_tensor("hc", [8, 3], F32, kind="ExternalInput")
    y_d = nc.dram_tensor("y", [8, bc], F32, kind="ExternalOutput")

    AMIN, AMAX, AADD = mybir.AluOpType.min, mybir.AluOpType.max, mybir.AluOpType.add

    with ExitStack() as ctx:
        tc = ctx.enter_context(tile.TileContext(nc))
        consts = ctx.enter_context(tc.tile_pool(name="consts", bufs=1))
        sbp = ctx.enter_context(tc.tile_pool(name="sbp", bufs=2))
        psp = ctx.enter_context(tc.tile_pool(name="psp", bufs=1, space="PSUM"))

        w1sb = consts.tile([128, 6 * 128], DT_MM)
        nc.sync.dma_start(out=w1sb, in_=w1_d[:, :])
        w2sb = consts.tile([128, 512], DT_MM)
        nc.sync.dma_start(out=w2sb, in_=w2_d[:, :])
        wrsb = consts.tile([128, 512], DT_MM)
        nc.sync.dma_start(out=wrsb, in_=wr_d[:, :])
        whsb = consts.tile([128, 16], DT_MM)
        nc.sync.dma_start(out=whsb, in_=wh_d[:, :])
        b2sb = consts.tile([128, 2], F32)
        nc.sync.dma_start(out=b2sb, in_=b2_d[:, :])
        brsb = consts.tile([128, 2], F32)
        nc.sync.dma_start(out=brsb, in_=br_d[:, :])
        hcsb = consts.tile([8, 3], F32)
        nc.sync.dma_start(out=hcsb, in_=hc_d[:, :])

        # GPSIMD cannot read PSUM, so phi1/phi2/rho relus go to ACT/DVE
        # (measured ~100-133G/s each on PSUM reads); GPSIMD gets the
        # SBUF-only pool adds and the final clip.
        def ph2_relu(eng, out_ap, in_ap, bias_ap):
            if eng == "act":
                nc.scalar.activation(out_ap, in_ap, RELU, bias=bias_ap)
            else:
                nc.vector.tensor_scalar(out_ap, in_ap, bias_ap, 0.0,
                                        op0=AADD, op1=AMAX)

        def finisher(pooled, s0):
            state = {}

            def s_rho(m):
                if "xs" not in state:
                    state["xs"] = sbp.tile([128, 2, 512], DT_MM, tag="xs",
                                           name="xs")
                xs = state["xs"]
                pr = psp.tile([128, 512], F32, tag="ph2", bufs=2, name="pr")
                nc.tensor.matmul(pr, wrsb[:, m * 128:(m + 1) * 128],
                                 pooled[:, 0:512], start=True, stop=False)
                nc.tensor.matmul(pr, wrsb[:, (2 + m) * 128:(3 + m) * 128],
                                 pooled[:, 512:1024], start=False, stop=True)
                nc.scalar.activation(xs[:, m, :], pr, RELU,
                                     bias=brsb[:, m:m + 1])

            def s_heads():
                xs = state["xs"]
                py = psp.tile([128, 512], F32, tag="ph2", bufs=2, name="py")
                nc.tensor.matmul(py[0:8, :], whsb[:, 0:8], xs[:, 0, :],
                                 start=True, stop=False)
                nc.tensor.matmul(py[0:8, :], whsb[:, 8:16], xs[:, 1, :],
                                 start=False, stop=True)
                tmp = sbp.tile([8, 512], F32, tag="tmp", name="tmp")
                nc.scalar.activation(tmp, py[0:8, :],
                                     mybir.ActivationFunctionType.Identity,
                                     bias=hcsb[:, 0:1])
                ysb = sbp.tile([8, 512], F32, tag="ysb", name="ysb")
                nc.gpsimd.tensor_scalar(ysb, tmp, hcsb[:, 1:2], hcsb[:, 2:3],
                                        op0=AMIN, op1=AMAX)
                nc.sync.dma_start(out=y_d[:, s0:s0 + 512], in_=ysb)

            return [lambda: s_rho(0), lambda: s_rho(1), s_heads]

        pending = []
        for t in range(nt):
            s0 = t * bt
            xgt = sbp.tile([128, 3, bt], DT_MM, tag="xgt", bufs=3)
            nc.sync.dma_start(out=xgt, in_=xg_d[:, :, s0:s0 + bt])
            # flat SBUF layouts (DVE bf16 2x mode requires flat APs):
            # hh[g] [128, 2048] = [A-k0 | A-k1 | B-k0 | B-k1]
            # rg[g] [128, 2048] = [A-m0 | A-m1 | B-m0 | B-m1]
            rg = [None] * 3
            sg = [None] * 3
            hh = [None] * 3
            final = sbp.tile([128, 2 * bt], DT_MM, tag="final")

            def phi1(g, m, eng):
                # two pairs row-tiled into one PE slot; per-m sub-round keeps
                # the psum rotation shallow (bufs=2 over 2-bank tiles)
                if m == 0:
                    hh[g] = sbp.tile([128, 2, 2 * bt], DT_MM, tag="h", bufs=3,
                                     name="h")
                    rg[g] = sbp.tile([128, 2, 2 * bt], DT_MM, tag="rg", bufs=3,
                                     name="rg")
                phm = psp.tile([128, 2, bt], F32, tag="ph1", bufs=2,
                               name="phm")
                c0 = (2 * g + m) * 128
                nc.tensor.matmul(phm[:, 0, :], w1sb[0:KG, c0:c0 + 128],
                                 xgt[0:KG, g, :], start=True, stop=True)
                nc.tensor.matmul(phm[:, 1, :], w1sb[KG:128, c0:c0 + 128],
                                 xgt[KG:128, g, :], start=True, stop=True)
                # both pairs' m-chunks in one op: out [128, 2(pair), bt]
                hv = hh[g][:, :, m * bt:(m + 1) * bt]
                if eng == "act":
                    nc.scalar.activation(hv, phm, RELU)
                else:
                    nc.vector.tensor_scalar_max(hv, phm, 0.0)

            def phi2(g, m, eng):
                # both pairs' m-chunks into one 2-bank psum tile -> one relu
                h = hh[g]
                ph2 = psp.tile([128, 2, bt], F32, tag="ph2", bufs=2,
                               name="ph2")
                for pair in range(2):
                    nc.tensor.matmul(ph2[:, pair, :],
                                     w2sb[:, m * 128:(m + 1) * 128],
                                     h[:, pair, 0:bt], start=True, stop=False)
                    nc.tensor.matmul(ph2[:, pair, :],
                                     w2sb[:, (2 + m) * 128:(3 + m) * 128],
                                     h[:, pair, bt:2 * bt],
                                     start=False, stop=True)
                ph2_relu(eng, rg[g][:, :, m * bt:(m + 1) * bt], ph2,
                         b2sb[:, m:m + 1])

            def add_sg(g, eng):
                # pair sum on flat APs (DVE bf16 2x mode needs flat)
                sg[g] = sbp.tile([128, 2 * bt], DT_MM, tag="sg", name="sg",
                                 bufs=3)
                eng.tensor_add(sg[g], rg[g][:, 0, :], rg[g][:, 1, :])

            def pend(i):
                if pending and i < len(pending):
                    pending[i]()

            # PE-ordered emission: every phi2 block is covered by independent
            # MM work (next phi1 sub-round / prev tile's rho+heads) while its
            # input relus complete on ACT/DVE in parallel.
            phi1(0, 0, "dve")
            phi1(0, 1, "act")
            pend(0)                    # prev rho m0
            pend(1)                    # prev rho m1
            phi1(1, 0, "dve")
            phi2(0, 0, "act")
            phi1(1, 1, "act")
            phi2(0, 1, "dve")
            pend(2)                    # prev heads
            phi2(1, 0, "act")
            add_sg(0, nc.gpsimd)
            phi1(2, 0, "dve")
            phi2(1, 1, "dve")
            phi1(2, 1, "act")
            add_sg(1, nc.gpsimd)
            t01 = sbp.tile([128, 2 * bt], DT_MM, tag="t01", name="t01")
            nc.gpsimd.tensor_add(t01, sg[0], sg[1])
            phi2(2, 0, "act")
            phi2(2, 1, "dve")
            add_sg(2, nc.vector)
            nc.vector.tensor_add(final, t01, sg[2])
            pending = finisher(final, s0)
        for stage in pending:
            stage()

    return nc


def _get_nc(bc, bt):
    key = (bc, bt)
    if key not in _CACHE:
        nc = _build_bass(bc, bt)
        nc.finalize()
        _CACHE[key] = nc
    return _CACHE[key]


def kernel(obs, ag, g, phi_w1, phi_b1, phi_w2, phi_b2,
           rho_w1, rho_b1, mean_w, mean_b, logstd_w, logstd_b):
    obs = np.asarray(obs, np.float32)
    ag = np.asarray(ag, np.float32)
    g = np.asarray(g, np.float32)
    B = obs.shape[0]
    assert B == B_FULL, f"kernel hardcoded for B={B_FULL}, got {B}"

    packed = _pack_weights(phi_w1, phi_b1, phi_w2, phi_b2, rho_w1, rho_b1,
                           mean_w, mean_b, logstd_w, logstd_b)
    xg = _pack_xg(obs, ag, g)

    nc = _get_nc(BC, BT)
    in_maps = []
    for c in range(N_CORES):
        m = dict(packed)
        m["xg"] = np.ascontiguousarray(xg[:, :, c * BC:(c + 1) * BC])
        in_maps.append(m)

    import os
    trace = bool(os.environ.get("KERNEL_TRACE"))
    res = run_bass_kernel_spmd(nc, in_maps, core_ids=list(range(N_CORES)),
                               trace=trace)
    global _last_results
    _last_results = res

    y = np.concatenate([res.results[c]["y"] for c in range(N_CORES)], axis=1)  # [8, B]
    out = np.ascontiguousarray(y.T)  # [B, 8]
    mean = out[:, 0:4].copy()
    logstd = out[:, 4:8].copy()
    return mean, logstd


_last_results = None


# revision 19
# speedup vs baseline: 1.6499x; 1.2315x over previous
"""Trainium2 Bass kernel for nn_ContinuousActor (GNN message passing actor MLP).

Strategy (pure data parallel over 8 cores, batch dim sharded):
  - Host repacks inputs feature-major: XT[74, B] = [obs.T; ag.T; g.T; ones].
    The ones row folds the (per-pair) phi1 bias into the matmul.
  - The per-pair input permutation/concat/one-hot of the reference is folded
    into 6 per-pair effective weight matrices W1e[p] of shape [74, 256]
    (host-side rearrangement of phi_w1 rows; one-hot rows fold into the bias).
  - On device everything is feature-major [features, batch]: per 512-col batch
    tile, 6x (phi1 matmul -> relu -> phi2 matmul -> relu+bias) then sum-pool,
    rho MLP, and the mean/logstd heads (with clip for logstd).
  - Matmuls run as float32r (full fp32 storage, fast PE path).
"""

import numpy as np
import ml_dtypes
from contextlib import ExitStack

import concourse.bass as bass
import concourse.mybir as mybir
import concourse.tile as tile
from concourse import bacc
from concourse.bass_utils import run_bass_kernel_spmd

F32 = mybir.dt.float32
F32R = mybir.dt.float32r
BF16 = mybir.dt.bfloat16
RELU = mybir.ActivationFunctionType.Relu

# matmul input dtype: BF16 (fast PE path w/ FWL) or F32R (accurate, ~2x slower)
DT_MM = BF16
DT_NP = ml_dtypes.bfloat16 if DT_MM == BF16 else np.float32

B_FULL = 65536
N_CORES = 8
BC = B_FULL // N_CORES  # 8192 batch rows per core
BT = 512                # batch tile (matmul free dim)
KG = 64                 # gathered rows per pair slot (53 used + pad)
NB_OBJ = 3
DIM_BODY = 10
DIM_OBJECT = 15
PERMS = [(0, 1), (0, 2), (1, 0), (1, 2), (2, 0), (2, 1)]
LOG_SIG_MIN, LOG_SIG_MAX = -20.0, 2.0

_CACHE = {}


def _pack_weights(phi_w1, phi_b1, phi_w2, phi_b2, rho_w1, rho_b1,
                  mean_w, mean_b, logstd_w, logstd_b):
    """Host-side weight repacking into device layouts (all float32)."""
    f = np.float32
    # phi1: gathered per-pair weights, row-tiled: col block (2g+m)*128 holds
    # pair 2g in rows 0:53 and pair 2g+1 in rows 64:117; row 52 carries bias.
    phi_w1 = np.asarray(phi_w1, f)
    w1 = np.zeros((128, 6 * 128), dtype=f)
    for p, (i, j) in enumerate(PERMS):
        g_, parity = p // 2, p % 2
        r0 = parity * KG
        W = np.zeros((53, 256), dtype=f)
        W[0:10] = phi_w1[12:22]        # body
        W[10:25] = phi_w1[25:40]       # obj_i features
        W[25:40] = phi_w1[43:58]       # obj_j features
        W[40:43] = phi_w1[0:3]         # ag_i
        W[43:46] = phi_w1[3:6]         # ag_j
        W[46:49] = phi_w1[6:9]         # g_i
        W[49:52] = phi_w1[9:12]        # g_j
        W[52] = phi_b1 + phi_w1[22 + i] + phi_w1[40 + j]  # bias + one-hots
        for m in range(2):
            w1[r0:r0 + 53, (2 * g_ + m) * 128:(2 * g_ + m + 1) * 128] = \
                W[:, m * 128:(m + 1) * 128]
    # phi2 / rho: [128, 4*128] with col block (2k+m) = W[k*128:(k+1)*128, m*128:(m+1)*128]
    def pack_256(w):
        out = np.empty((128, 512), dtype=f)
        for k in range(2):
            for m in range(2):
                out[:, (2 * k + m) * 128:(2 * k + m + 1) * 128] = \
                    w[k * 128:(k + 1) * 128, m * 128:(m + 1) * 128]
        return out
    w2 = pack_256(np.asarray(phi_w2, f))
    wr = pack_256(np.asarray(rho_w1, f))
    b2 = np.asarray(phi_b2, f).reshape(2, 128).T.copy()   # [128, 2], col m
    br = np.asarray(rho_b1, f).reshape(2, 128).T.copy()
    # heads: [128, 16], col block k*8 = Wh[k*128:(k+1)*128, :]
    wh_full = np.concatenate([np.asarray(mean_w, f), np.asarray(logstd_w, f)], axis=1)  # [256, 8]
    wh = np.concatenate([wh_full[0:128, :], wh_full[128:256, :]], axis=1)  # [128, 16]
    bh = np.concatenate([np.asarray(mean_b, f), np.asarray(logstd_b, f)]).reshape(1, 8)
    w1, w2, wr, wh, bh = (a.astype(DT_NP) for a in (w1, w2, wr, wh, bh))
    # per-partition clip bounds for the 8 head rows: mean rows unclipped
    big = np.float32(3.0e38)
    clipb = np.empty((8, 2), dtype=f)
    clipb[0:4, 0], clipb[4:8, 0] = big, LOG_SIG_MAX   # hi (min op)
    clipb[0:4, 1], clipb[4:8, 1] = -big, LOG_SIG_MIN  # lo (max op)
    ones = np.ones((1, BT), dtype=DT_NP)
    return dict(w1=w1, w2=w2, b2=b2, wr=wr, br=br, wh=wh, bh=bh, clipb=clipb,
                ones=ones)


def _pack_xt(obs, ag, g):
    """Gathered per-pair inputs, row-tiled: xg[128, 3, B]; rows 0:53 = pair
    2gi, rows 64:117 = pair 2gi+1; row 52/116 = ones (bias row)."""
    B = obs.shape[0]
    obsT = obs.T.astype(DT_NP)
    agT = ag.T.astype(DT_NP)
    gT = g.T.astype(DT_NP)
    xg = np.zeros((128, 3, B), dtype=DT_NP)
    for p, (i, j) in enumerate(PERMS):
        gi, parity = p // 2, p % 2
        r0 = parity * KG
        blk = xg[:, gi]
        blk[r0 + 0:r0 + 10] = obsT[0:10]
        blk[r0 + 10:r0 + 25] = obsT[10 + 15 * i:25 + 15 * i]
        blk[r0 + 25:r0 + 40] = obsT[10 + 15 * j:25 + 15 * j]
        blk[r0 + 40:r0 + 43] = agT[3 * i:3 * i + 3]
        blk[r0 + 43:r0 + 46] = agT[3 * j:3 * j + 3]
        blk[r0 + 46:r0 + 49] = gT[3 * i:3 * i + 3]
        blk[r0 + 49:r0 + 52] = gT[3 * j:3 * j + 3]
        blk[r0 + 52] = np.asarray(1.0, DT_NP)
    return xg


def _build_bass(bc, bt):
    """Build the per-core Bass program for a core batch of `bc` rows, tiled by `bt`."""
    nt = bc // bt
    nc = bacc.Bacc(trn_type="TRN2")

    xt_d = nc.dram_tensor("xt", [128, 3, bc], DT_MM, kind="ExternalInput")
    w1_d = nc.dram_tensor("w1", [128, 6 * 128], DT_MM, kind="ExternalInput")
    w2_d = nc.dram_tensor("w2", [128, 512], DT_MM, kind="ExternalInput")
    b2_d = nc.dram_tensor("b2", [128, 2], F32, kind="ExternalInput")
    wr_d = nc.dram_tensor("wr", [128, 512], DT_MM, kind="ExternalInput")
    br_d = nc.dram_tensor("br", [128, 2], F32, kind="ExternalInput")
    wh_d = nc.dram_tensor("wh", [128, 16], DT_MM, kind="ExternalInput")
    bh_d = nc.dram_tensor("bh", [1, 8], DT_MM, kind="ExternalInput")
    clipb_d = nc.dram_tensor("clipb", [8, 2], F32, kind="ExternalInput")
    ones_d = nc.dram_tensor("ones", [1, bt], DT_MM, kind="ExternalInput")
    y_d = nc.dram_tensor("y", [8, bc], F32, kind="ExternalOutput")

    with ExitStack() as ctx:
        tc = ctx.enter_context(tile.TileContext(nc))
        consts = ctx.enter_context(tc.tile_pool(name="consts", bufs=1))
        sbp = ctx.enter_context(tc.tile_pool(name="sbp", bufs=3))
        psp = ctx.enter_context(tc.tile_pool(name="psp", bufs=2, space="PSUM"))

        w1sb = consts.tile([128, 6 * 128], DT_MM)
        nc.sync.dma_start(out=w1sb, in_=w1_d[:, :])
        w2sb = consts.tile([128, 512], DT_MM)
        nc.sync.dma_start(out=w2sb, in_=w2_d[:, :])
        wrsb = consts.tile([128, 512], DT_MM)
        nc.sync.dma_start(out=wrsb, in_=wr_d[:, :])
        whsb = consts.tile([128, 16], DT_MM)
        nc.sync.dma_start(out=whsb, in_=wh_d[:, :])
        b2sb = consts.tile([128, 2], F32)
        nc.sync.dma_start(out=b2sb, in_=b2_d[:, :])
        brsb = consts.tile([128, 2], F32)
        nc.sync.dma_start(out=brsb, in_=br_d[:, :])
        bhsb = consts.tile([1, 8], DT_MM)
        nc.sync.dma_start(out=bhsb, in_=bh_d[:, :])
        clipsb = consts.tile([8, 2], F32)
        nc.sync.dma_start(out=clipsb, in_=clipb_d[:, :])
        ones_sb = consts.tile([1, bt], DT_MM)
        nc.sync.dma_start(out=ones_sb, in_=ones_d[:, :])

        AMIN, AMAX, AADD = mybir.AluOpType.min, mybir.AluOpType.max, mybir.AluOpType.add

        def finisher(acc, s0):
            # rho + heads + clip + store for one tile, split into 3 stages
            # emitted between the next tile's pairs so no engine stream stalls
            state = {}

            def stage_a():  # rho matmuls
                pr = psp.tile([128, 2 * bt], F32, tag="pr", name="pr", bufs=1)
                for m in range(2):
                    for k in range(2):
                        nc.tensor.matmul(
                            pr[:, m * bt:(m + 1) * bt],
                            wrsb[:, (2 * k + m) * 128:(2 * k + m + 1) * 128],
                            acc[:, k * bt:(k + 1) * bt],
                            start=(k == 0), stop=(k == 1),
                        )
                state["pr"] = pr

            def stage_b():  # rho relu (split across DVE/ACT)
                pr = state["pr"]
                xs = sbp.tile([128, 2 * bt], DT_MM, tag="xs", name="xs")
                nc.vector.tensor_scalar(
                    xs[:, 0:bt], pr[:, 0:bt],
                    brsb[:, 0:1], 0.0, op0=AADD, op1=AMAX,
                )
                nc.scalar.activation(
                    xs[:, bt:2 * bt], pr[:, bt:2 * bt],
                    RELU, bias=brsb[:, 1:2],
                )
                state["xs"] = xs

            def stage_c():  # heads + clip + store
                xs = state["xs"]
                py = psp.tile([8, bt], F32, tag="ph1", name="py")
                for k in range(2):
                    nc.tensor.matmul(
                        py, whsb[:, k * 8:(k + 1) * 8], xs[:, k * bt:(k + 1) * bt],
                        start=(k == 0), stop=False,
                    )
                nc.tensor.matmul(py, bhsb, ones_sb, start=False, stop=True)
                ysb = sbp.tile([8, bt], F32, tag="ysb", name="ysb")
                nc.vector.tensor_scalar(
                    ysb, py, clipsb[:, 0:1], clipsb[:, 1:2],
                    op0=AMIN, op1=AMAX,
                )
                nc.sync.dma_start(out=y_d[:, s0:s0 + bt], in_=ysb)

            return [stage_a, stage_b, stage_c]

        pending = None
        for t in range(nt):
            s0 = t * bt
            xts = sbp.tile([128, 3, bt], DT_MM, tag="xts")
            nc.sync.dma_start(out=xts, in_=xt_d[:, :, s0:s0 + bt])

            acc = None
            ph1_pair = [None, None]
            for p in range(6):
                if p % 2 == 0:
                    # row-tiled g-round: both pairs' phi1 matmuls emitted
                    # adjacently; (A,m) and (B,m) run concurrently on the PE
                    # (row groups 0:64 / 64:128 -> 2 slots instead of 4)
                    g_ = p // 2
                    ph1_pair[0] = psp.tile([128, 2 * bt], F32, tag="ph1",
                                           name="ph1a")
                    ph1_pair[1] = psp.tile([128, 2 * bt], F32, tag="ph1",
                                           name="ph1b")
                    for m in range(2):
                        c0 = (2 * g_ + m) * 128
                        nc.tensor.matmul(
                            ph1_pair[0][:, m * bt:(m + 1) * bt],
                            w1sb[0:KG, c0:c0 + 128],
                            xts[0:KG, g_, :], start=True, stop=True)
                        nc.tensor.matmul(
                            ph1_pair[1][:, m * bt:(m + 1) * bt],
                            w1sb[KG:128, c0:c0 + 128],
                            xts[KG:128, g_, :], start=True, stop=True)
                ph1 = ph1_pair[p % 2]
                h1 = sbp.tile([128, 2 * bt], DT_MM, tag="h1")
                if p % 3 == 0:
                    nc.vector.tensor_scalar_max(h1, ph1, 0.0)  # relu (DVE)
                else:
                    nc.scalar.activation(h1, ph1, RELU)        # relu (ACT)

                # phi2: per-(p,m) single-bank psum units for deep pipelining
                ph2s = []
                for m in range(2):
                    ph2 = psp.tile([128, bt], F32, tag="ph2m", name="ph2")
                    for k in range(2):
                        nc.tensor.matmul(
                            ph2,
                            w2sb[:, (2 * k + m) * 128:(2 * k + m + 1) * 128],
                            h1[:, k * bt:(k + 1) * bt],
                            start=(k == 0), stop=(k == 1),
                        )
                    ph2s.append(ph2)
                if p == 0:
                    r = sbp.tile([128, 2 * bt], DT_MM, tag="acc")
                    acc = r
                else:
                    r = sbp.tile([128, 2 * bt], DT_MM, tag="rtmp")
                for m in range(2):
                    if (2 * p + m) % 3 == 0:
                        nc.vector.tensor_scalar(
                            r[:, m * bt:(m + 1) * bt], ph2s[m],
                            b2sb[:, m:m + 1], 0.0, op0=AADD, op1=AMAX,
                        )
                    else:
                        nc.scalar.activation(
                            r[:, m * bt:(m + 1) * bt], ph2s[m],
                            RELU, bias=b2sb[:, m:m + 1],
                        )
                if p >= 1:
                    # accumulate in-place; hidden behind the next pair's work
                    eng = nc.gpsimd if p in (2, 4) else nc.vector
                    eng.tensor_add(acc, acc, r)
                if pending and 0 <= p - 1 < len(pending):
                    pending[p - 1]()  # prev tile's rho/heads staged at pairs 1-3
            pending = finisher(acc, s0)
        for stage in pending:
            stage()

    return nc


def _get_nc(bc, bt):
    key = (bc, bt)
    if key not in _CACHE:
        nc = _build_bass(bc, bt)
        nc.finalize()  # Bacc: run compile passes (wait-splitting, reg alloc)
        _CACHE[key] = nc
    return _CACHE[key]


def kernel(obs, ag, g, phi_w1, phi_b1, phi_w2, phi_b2,
           rho_w1, rho_b1, mean_w, mean_b, logstd_w, logstd_b):
    obs = np.asarray(obs, np.float32)
    ag = np.asarray(ag, np.float32)
    g = np.asarray(g, np.float32)
    B = obs.shape[0]
    assert B == B_FULL, f"kernel hardcoded for B={B_FULL}, got {B}"

    packed = _pack_weights(phi_w1, phi_b1, phi_w2, phi_b2, rho_w1, rho_b1,
                           mean_w, mean_b, logstd_w, logstd_b)
    xt = _pack_xt(obs, ag, g)

    nc = _get_nc(BC, BT)
    in_maps = []
    for c in range(N_CORES):
        m = dict(packed)
        m["xt"] = np.ascontiguousarray(xt[:, :, c * BC:(c + 1) * BC])
        in_maps.append(m)

    import os
    trace = bool(os.environ.get("KERNEL_TRACE"))
    res = run_bass_kernel_spmd(nc, in_maps, core_ids=list(range(N_CORES)),
                               trace=trace)
    global _last_results
    _last_results = res

    y = np.concatenate([res.results[c]["y"] for c in range(N_CORES)], axis=1)  # [8, B]
    out = np.ascontiguousarray(y.T)  # [B, 8]
    mean = out[:, 0:4].copy()
    logstd = out[:, 4:8].copy()
    return mean, logstd


_last_results = None

